# revision 1
# baseline (speedup 1.0000x reference)
"""Trainium2 Bass kernel for ComplexDFT256.

Math: out[b, 0:256]   = x_real @ cos.T - x_imag @ sin.T
      out[b, 256:512] = x_imag @ cos.T + x_real @ sin.T
which is a single fused matmul  out[B,512] = Z[B,512] @ M[512,512]
with Z = [x_real | x_imag] and M = [[cos.T, sin.T], [-sin.T, cos.T]].

Sharding: pure data parallel over batch across 8 NeuronCores (8192 rows
each). Host pre-transposes Z to [512, B] so the contraction dim lands on
SBUF partitions with perfectly contiguous DMA, and pre-rounds operands
to fp32r (fp32 with 11-bit mantissa; PE runs fp32r at full rate vs 4x
slower for fp32). PSUM accumulates in full fp32. Measured end-to-end
error vs fp64 reference ~1.6e-4 of output scale.
"""
import numpy as np

import concourse.bacc as bacc
import concourse.mybir as mybir
import concourse.tile as tile
from concourse.bass_utils import run_bass_kernel_spmd

N_CORES = 8
BATCH = 65536
FFT = 256
C = 2 * FFT            # contraction dim = 512
J = 2 * FFT            # output features = 512
B_SHARD = BATCH // N_CORES   # 8192
CHUNK_B = 1024         # batch rows loaded per DMA chunk
N_CHUNKS = B_SHARD // CHUNK_B
SUB_PER_CHUNK = CHUNK_B // 128
N_K = C // 128         # 4 contraction tiles

_cache = {}


def _round_fp32r(a: np.ndarray) -> np.ndarray:
    """Round fp32 to fp32r (11-bit mantissa, round-nearest-even).

    Matches neuronxcc static_cast_fp32_to_fp32r bit-exactly; required
    because the BIR verifier insists fp32r matmul inputs are pre-rounded.
    """
    bits = np.ascontiguousarray(a).view(np.uint32)
    lsb = (bits >> 12) & 1
    out = ((bits.astype(np.uint64) + 0x7FF + lsb) & 0xFFFFF000).astype(np.uint32)
    return out.view(np.float32).reshape(a.shape)


def _build_nc(reps: int = 1):
    nc = bacc.Bacc("TRN2", target_bir_lowering=False, debug=False,
                   num_devices=N_CORES)
    f32 = mybir.dt.float32
    f32r = mybir.dt.float32r

    zt_dram = nc.dram_tensor("zt", [C, B_SHARD], f32r, kind="ExternalInput")
    m_dram = nc.dram_tensor("m", [C, J], f32r, kind="ExternalInput")
    out_dram = nc.dram_tensor("out", [B_SHARD, J], f32, kind="ExternalOutput")

    with tile.TileContext(nc) as tc:
        with (
            tc.tile_pool(name="mpool", bufs=1) as mpool,
            tc.tile_pool(name="zpool", bufs=4) as zpool,
            tc.tile_pool(name="opool", bufs=8) as opool,
            tc.tile_pool(name="psum", bufs=6, space="PSUM") as psum_pool,
        ):
            m_sb = []
            for k in range(N_K):
                mt = mpool.tile([128, J], f32r, tag=f"m{k}")
                # SWDGE: keeps the m loads off the SP queue so the first
                # zt chunk streams in parallel
                nc.gpsimd.dma_start(mt[:], m_dram[k * 128:(k + 1) * 128, :])
                m_sb.append(mt)

            def body():
                for i in range(N_CHUNKS):
                    zt_sb = zpool.tile([128, N_K, CHUNK_B], f32r, tag="zt")
                    for k in range(N_K):
                        nc.sync.dma_start(
                            zt_sb[:, k, :],
                            zt_dram[k * 128:(k + 1) * 128,
                                    i * CHUNK_B:(i + 1) * CHUNK_B],
                        )
                    for j in range(SUB_PER_CHUNK):
                        acc = psum_pool.tile([128, J], f32, tag="acc")
                        for k in range(N_K):
                            nc.tensor.matmul(
                                acc[:],
                                zt_sb[:, k, j * 128:(j + 1) * 128],
                                m_sb[k][:],
                                start=(k == 0), stop=(k == N_K - 1),
                            )
                        out_sb = opool.tile([128, J], f32, tag="out")
                        t = i * SUB_PER_CHUNK + j
                        # copies on DVE only; stores issue from the ACT HWDGE
                        # queue so they never head-of-line-block the SP loads
                        nc.vector.tensor_copy(out_sb[:], acc[:])
                        nc.scalar.dma_start(
                            out_dram[t * 128:(t + 1) * 128, :], out_sb[:])

            if reps == 1:
                body()
            else:
                with tc.For_i(0, reps, 1):
                    body()

    nc.compile()
    return nc


def _get_nc():
    if "nc" not in _cache:
        _cache["nc"] = _build_nc()
    return _cache["nc"]


def _prepare_in_maps(x, cos_kernel, sin_kernel):
    x = np.asarray(x, dtype=np.float32)
    cos = np.asarray(cos_kernel, dtype=np.float32)
    sin = np.asarray(sin_kernel, dtype=np.float32)

    m = np.empty((C, J), dtype=np.float32)
    m[:FFT, :FFT] = cos.T
    m[:FFT, FFT:] = sin.T
    m[FFT:, :FFT] = -sin.T
    m[FFT:, FFT:] = cos.T
    m_r = _round_fp32r(m)

    z = _round_fp32r(x.reshape(BATCH, C))
    in_maps = []
    for c in range(N_CORES):
        shard = np.ascontiguousarray(
            z[c * B_SHARD:(c + 1) * B_SHARD, :].T)  # [C, B_SHARD]
        in_maps.append({"zt": shard, "m": m_r})
    return in_maps


def _run(in_maps, trace=False):
    nc = _get_nc()
    return run_bass_kernel_spmd(nc, in_maps, list(range(N_CORES)), trace=trace)


def kernel(x, cos_kernel, sin_kernel):
    in_maps = _prepare_in_maps(x, cos_kernel, sin_kernel)
    res = _run(in_maps)
    out = np.concatenate([r["out"] for r in res.results], axis=0)
    return out.reshape(BATCH, J, 1)



# revision 4
# speedup vs baseline: 1.3452x; 1.3452x over previous
"""Trainium2 Bass kernel for ComplexDFT256 — bf16 radix-2 version.

Math: the 256-point complex DFT out = z @ M (z = [xr | xi], M the
512x512 real form of the DFT) is split radix-2 over time samples:
  X[k]      = E[k] + G[k]        k = 0..127
  X[k+128]  = E[k] - G[k]
with E = DFT-128 of even samples and G = (twiddle * DFT-128) of odd
samples; the twiddles fold into G's matrix on the host, so on-device
this is two [B,256]@[256,256] matmuls (half the PE work of the dense
form) plus one add + one sub per output tile (DVE).

Everything streams in bf16 (inputs pre-cast on host, outputs cast by
the DVE butterfly, PSUM accumulates fp32), halving HBM traffic vs
fp32r: 8 MB in + 8 MB out per core.  Measured end-to-end error vs the
fp32 reference ~2.7e-3 of output norm (tolerance 2e-2).

Sharding: pure data parallel over batch across 8 NeuronCores (8192
rows each).  Host pre-permutes columns to [even | odd] order and
transposes to [512, B] so the contraction dim lands on SBUF partitions
with contiguous DMA.

Output leaves the device in a [64, 128, 2, 512] layout (one DMA per
256-row group, partition-major) and is un-permuted on the host.
"""
import numpy as np
import ml_dtypes

import concourse.bacc as bacc
import concourse.mybir as mybir
import concourse.tile as tile
from concourse.bass_utils import run_bass_kernel_spmd

N_CORES = 8
BATCH = 65536
FFT = 256
C = 2 * FFT            # contraction dim = 512 ([even 256 | odd 256])
J = 2 * FFT            # output features = 512
B_SHARD = BATCH // N_CORES   # 8192
CHUNK_B = 2048         # batch rows loaded per DMA chunk
N_CHUNKS = B_SHARD // CHUNK_B
TILES_PER_CHUNK = CHUNK_B // 128          # 16
GROUPS_PER_CHUNK = TILES_PER_CHUNK // 2   # butterfly group = 2 tiles
N_GROUPS = B_SHARD // 256                 # 32

BF16 = ml_dtypes.bfloat16

_cache = {}


def _build_nc(reps: int = 1):
    nc = bacc.Bacc("TRN2", target_bir_lowering=False, debug=False,
                   num_devices=N_CORES)
    f32 = mybir.dt.float32
    bf16 = mybir.dt.bfloat16

    zt_dram = nc.dram_tensor("zt", [C, B_SHARD], bf16, kind="ExternalInput")
    m_dram = nc.dram_tensor("m", [C, 256], bf16, kind="ExternalInput")
    # [group, partition, tile-in-group, col]; host un-permutes
    out_dram = nc.dram_tensor("out", [N_GROUPS, 128, 2, J], bf16,
                              kind="ExternalOutput")

    with tile.TileContext(nc) as tc:
        with (
            tc.tile_pool(name="mpool", bufs=1) as mpool,
            tc.tile_pool(name="zpool", bufs=3) as zpool,
            tc.tile_pool(name="gpool", bufs=4) as gpool,
            tc.tile_pool(name="opool", bufs=6) as opool,
            tc.tile_pool(name="psum", bufs=4, space="PSUM") as psum_pool,
        ):
            m_sb = []
            for k in range(4):
                mt = mpool.tile([128, 256], bf16, tag=f"m{k}")
                # SWDGE: keeps the m loads off the SP queue so the first
                # zt chunk streams in parallel
                nc.gpsimd.dma_start(mt[:], m_dram[k * 128:(k + 1) * 128, :])
                m_sb.append(mt)

            def body():
                for i in range(N_CHUNKS):
                    zt_sb = zpool.tile([128, 4, CHUNK_B], bf16, tag="zt")
                    for k in range(4):
                        nc.sync.dma_start(
                            zt_sb[:, k, :],
                            zt_dram[k * 128:(k + 1) * 128,
                                    i * CHUNK_B:(i + 1) * CHUNK_B],
                        )
                    for g in range(GROUPS_PER_CHUNK):
                        # P[:, 2s+q, h, :]: tile s, q=0 even-half (E),
                        # q=1 odd-half (G), h = lo/hi 128 output cols
                        P = psum_pool.tile([128, 4, 2, 128], f32, tag="acc")
                        for s in range(2):
                            t = g * 2 + s
                            bsl = slice(t * 128, (t + 1) * 128)
                            nc.tensor.matmul(
                                P[:, 2 * s, :, :],
                                zt_sb[:, 0, bsl], m_sb[0][:],
                                start=True, stop=False)
                            nc.tensor.matmul(
                                P[:, 2 * s, :, :],
                                zt_sb[:, 1, bsl], m_sb[1][:],
                                start=False, stop=True)
                            nc.tensor.matmul(
                                P[:, 2 * s + 1, :, :],
                                zt_sb[:, 2, bsl], m_sb[2][:],
                                start=True, stop=False)
                            nc.tensor.matmul(
                                P[:, 2 * s + 1, :, :],
                                zt_sb[:, 3, bsl], m_sb[3][:],
                                start=False, stop=True)
                        # out_sb[:, s, q, h, :]: X[k] = E+G (h=0),
                        # X[k+128] = E-G (h=1), q = Re/Im half
                        out_sb = opool.tile([128, 2, 2, 2, 128], bf16,
                                            tag="out")
                        te = P[:, 0::2, :, :]
                        tg = P[:, 1::2, :, :]
                        # DVE TensorTensor may read only one PSUM input;
                        # stage G through SBUF on the otherwise-idle ACT
                        tgc = gpool.tile([128, 2, 2, 128], f32, tag="tgc")
                        nc.scalar.copy(tgc[:], tg)
                        nc.vector.tensor_add(out_sb[:, :, :, 0, :], te, tgc[:])
                        nc.vector.tensor_sub(out_sb[:, :, :, 1, :], te, tgc[:])
                        G = i * GROUPS_PER_CHUNK + g
                        # split stores across the ACT and SP HWDGE queues
                        eng = nc.scalar if (g % 2 == 0) else nc.sync
                        eng.dma_start(out_dram[G], out_sb[:])

            if reps == 1:
                body()
            else:
                with tc.For_i(0, reps, 1):
                    body()

    nc.compile()
    return nc


def _get_nc():
    if "nc" not in _cache:
        _cache["nc"] = _build_nc()
    return _cache["nc"]


def _prepare_in_maps(x, cos_kernel, sin_kernel):
    x = np.asarray(x, dtype=np.float32)
    cos = np.asarray(cos_kernel, dtype=np.float32)
    sin = np.asarray(sin_kernel, dtype=np.float32)

    m = np.empty((C, J), dtype=np.float32)
    m[:FFT, :FFT] = cos.T
    m[:FFT, FFT:] = sin.T
    m[FFT:, :FFT] = -sin.T
    m[FFT:, FFT:] = cos.T

    # radix-2: even/odd sample rows; cols k<128 of both Re and Im halves
    # (cols k+128 equal these up to the sign of the odd-row block)
    rows_e = np.concatenate([np.arange(0, 256, 2), np.arange(256, 512, 2)])
    rows_o = rows_e + 1
    cols_lo = np.concatenate([np.arange(0, 128), np.arange(256, 384)])
    me = m[np.ix_(rows_e, cols_lo)]     # [256, 256]
    mg = m[np.ix_(rows_o, cols_lo)]     # [256, 256]
    m_dev = np.concatenate([me, mg], axis=0).astype(BF16)  # [512, 256]

    z = x.reshape(BATCH, C)[:, np.concatenate([rows_e, rows_o])].astype(BF16)
    zt = np.ascontiguousarray(z.view(np.uint16).T)  # [512, BATCH] as u16

    in_maps = []
    for c in range(N_CORES):
        shard = np.ascontiguousarray(
            zt[:, c * B_SHARD:(c + 1) * B_SHARD]).view(BF16)
        in_maps.append({"zt": shard, "m": m_dev})
    return in_maps


def _run(in_maps, trace=False):
    nc = _get_nc()
    return run_bass_kernel_spmd(nc, in_maps, list(range(N_CORES)), trace=trace)


def kernel(x, cos_kernel, sin_kernel):
    in_maps = _prepare_in_maps(x, cos_kernel, sin_kernel)
    res = _run(in_maps)
    outs = []
    for r in res.results:
        o = np.asarray(r["out"])  # [64, 128, 2, 512] bf16
        o = o.view(np.uint16).transpose(0, 2, 1, 3).reshape(B_SHARD, J)
        outs.append(o)
    out = np.concatenate(outs, axis=0).view(BF16).astype(np.float32)
    return out.reshape(BATCH, J, 1)


# revision 9
# speedup vs baseline: 1.4038x; 1.0436x over previous
"""Trainium2 Bass kernel for ComplexDFT256 — bf16 radix-2 version.

Math: the 256-point complex DFT out = z @ M (z = [xr | xi], M the
512x512 real form of the DFT) is split radix-2 over time samples:
  X[k]      = E[k] + G[k]        k = 0..127
  X[k+128]  = E[k] - G[k]
with E = DFT-128 of even samples and G = (twiddle * DFT-128) of odd
samples; the twiddles fold into G's matrix on the host, so on-device
this is two [B,256]@[256,256] matmuls (half the PE work of the dense
form) plus one add + one sub per output tile (DVE).

Everything streams in bf16 (inputs pre-cast on host, outputs cast by
the DVE butterfly, PSUM accumulates fp32), halving HBM traffic vs
fp32r: 8 MB in + 8 MB out per core.  Measured end-to-end error vs the
fp32 reference ~2.7e-3 of output norm (tolerance 2e-2).

Sharding: pure data parallel over batch across 8 NeuronCores (8192
rows each).  Host pre-permutes columns to [even | odd] order and
transposes to [512, B] so the contraction dim lands on SBUF partitions
with contiguous DMA.

Output leaves the device in a [64, 128, 2, 512] layout (one DMA per
256-row group, partition-major) and is un-permuted on the host.
"""
import numpy as np
import ml_dtypes

import concourse.bacc as bacc
import concourse.mybir as mybir
import concourse.tile as tile
from concourse.bass_utils import run_bass_kernel_spmd

N_CORES = 8
BATCH = 65536
FFT = 256
C = 2 * FFT            # contraction dim = 512 ([even 256 | odd 256])
J = 2 * FFT            # output features = 512
B_SHARD = BATCH // N_CORES   # 8192
CHUNK_B = 2048         # batch rows loaded per DMA chunk
N_CHUNKS = B_SHARD // CHUNK_B
TILES_PER_CHUNK = CHUNK_B // 128          # 16
GROUPS_PER_CHUNK = TILES_PER_CHUNK // 2   # butterfly group = 2 tiles
N_GROUPS = B_SHARD // 256                 # 32

BF16 = ml_dtypes.bfloat16

_cache = {}


def _build_nc(reps: int = 1, unroll: bool = False):
    nc = bacc.Bacc("TRN2", target_bir_lowering=False, debug=False,
                   num_devices=N_CORES)
    f32 = mybir.dt.float32
    bf16 = mybir.dt.bfloat16

    zt_dram = nc.dram_tensor("zt", [C, B_SHARD], bf16, kind="ExternalInput")
    m_dram = nc.dram_tensor("m", [C, 256], bf16, kind="ExternalInput")
    # [group, partition, lo/hi, tile-in-group, 256]; host un-permutes
    out_dram = nc.dram_tensor("out", [N_GROUPS, 128, 2, 2, 256], bf16,
                              kind="ExternalOutput")

    with tile.TileContext(nc) as tc:
        with (
            tc.tile_pool(name="mpool", bufs=1) as mpool,
            tc.tile_pool(name="zpool", bufs=3) as zpool,
            tc.tile_pool(name="gpool", bufs=4) as gpool,
            tc.tile_pool(name="opool", bufs=6) as opool,
            tc.tile_pool(name="psum", bufs=4, space="PSUM") as psum_pool,
        ):
            m_sb = []
            for k in range(4):
                mt = mpool.tile([128, 256], bf16, tag=f"m{k}")
                # SWDGE: keeps the m loads off the SP queue so the first
                # zt chunk streams in parallel
                nc.gpsimd.dma_start(mt[:], m_dram[k * 128:(k + 1) * 128, :])
                m_sb.append(mt)

            def body():
                for i in range(N_CHUNKS):
                    zt_sb = zpool.tile([128, 4, CHUNK_B], bf16, tag="zt")
                    for k in range(4):
                        nc.sync.dma_start(
                            zt_sb[:, k, :],
                            zt_dram[k * 128:(k + 1) * 128,
                                    i * CHUNK_B:(i + 1) * CHUNK_B],
                        )
                    for g in range(GROUPS_PER_CHUNK):
                        # P[:, 2s+q, :]: tile s, q=0 even-half (E),
                        # q=1 odd-half (G); 256 cols = [Re k<128 | Im k<128]
                        P = psum_pool.tile([128, 4, 256], f32, tag="acc")
                        for s in range(2):
                            t = g * 2 + s
                            bsl = slice(t * 128, (t + 1) * 128)
                            nc.tensor.matmul(
                                P[:, 2 * s, :],
                                zt_sb[:, 0, bsl], m_sb[0][:],
                                start=True, stop=False)
                            nc.tensor.matmul(
                                P[:, 2 * s, :],
                                zt_sb[:, 1, bsl], m_sb[1][:],
                                start=False, stop=True)
                            nc.tensor.matmul(
                                P[:, 2 * s + 1, :],
                                zt_sb[:, 2, bsl], m_sb[2][:],
                                start=True, stop=False)
                            nc.tensor.matmul(
                                P[:, 2 * s + 1, :],
                                zt_sb[:, 3, bsl], m_sb[3][:],
                                start=False, stop=True)
                        # one contiguous PSUM->SBUF drain on ACT (bf16),
                        # so both DVE butterfly ops run all-SBUF in bf16
                        # (DVE TensorTensor may read only one PSUM input,
                        # and all-SBUF 16-bit ops unlock DVE 2x mode)
                        stg = gpool.tile([128, 2, 2, 256], bf16, tag="stg")
                        nc.scalar.copy(stg[:], P[:])
                        te = stg[:, :, 0, :]
                        tg = stg[:, :, 1, :]
                        # out_sb[:, a, s, :]: a=0 -> X[k]=E+G, a=1 ->
                        # X[k+128]=E-G; contiguous 512-elem DVE writes
                        out_sb = opool.tile([128, 2, 2, 256], bf16,
                                            tag="out")
                        nc.vector.tensor_add(out_sb[:, 0, :, :], te, tg)
                        nc.vector.tensor_sub(out_sb[:, 1, :, :], te, tg)
                        G = i * GROUPS_PER_CHUNK + g
                        # stores on the SP HWDGE queue with the loads;
                        # ACT stays a pure PSUM-drain engine
                        nc.sync.dma_start(out_dram[G], out_sb[:])

            if reps == 1:
                body()
            elif unroll:
                for _ in range(reps):
                    body()
            else:
                with tc.For_i(0, reps, 1):
                    body()

    nc.compile()
    return nc


def _get_nc():
    if "nc" not in _cache:
        _cache["nc"] = _build_nc()
    return _cache["nc"]


def _prepare_in_maps(x, cos_kernel, sin_kernel):
    x = np.asarray(x, dtype=np.float32)
    cos = np.asarray(cos_kernel, dtype=np.float32)
    sin = np.asarray(sin_kernel, dtype=np.float32)

    m = np.empty((C, J), dtype=np.float32)
    m[:FFT, :FFT] = cos.T
    m[:FFT, FFT:] = sin.T
    m[FFT:, :FFT] = -sin.T
    m[FFT:, FFT:] = cos.T

    # radix-2: even/odd sample rows; cols k<128 of both Re and Im halves
    # (cols k+128 equal these up to the sign of the odd-row block)
    rows_e = np.concatenate([np.arange(0, 256, 2), np.arange(256, 512, 2)])
    rows_o = rows_e + 1
    cols_lo = np.concatenate([np.arange(0, 128), np.arange(256, 384)])
    me = m[np.ix_(rows_e, cols_lo)]     # [256, 256]
    mg = m[np.ix_(rows_o, cols_lo)]     # [256, 256]
    m_dev = np.concatenate([me, mg], axis=0).astype(BF16)  # [512, 256]

    z = x.reshape(BATCH, C)[:, np.concatenate([rows_e, rows_o])].astype(BF16)
    zt = np.ascontiguousarray(z.view(np.uint16).T)  # [512, BATCH] as u16

    in_maps = []
    for c in range(N_CORES):
        shard = np.ascontiguousarray(
            zt[:, c * B_SHARD:(c + 1) * B_SHARD]).view(BF16)
        in_maps.append({"zt": shard, "m": m_dev})
    return in_maps


def _run(in_maps, trace=False):
    nc = _get_nc()
    return run_bass_kernel_spmd(nc, in_maps, list(range(N_CORES)), trace=trace)


def kernel(x, cos_kernel, sin_kernel):
    in_maps = _prepare_in_maps(x, cos_kernel, sin_kernel)
    res = _run(in_maps)
    outs = []
    for r in res.results:
        # [G, p, a, s, 256] bf16; row = G*256 + s*128 + p,
        # col = q*256 + a*128 + k with the last dim = (q, k)
        o = np.asarray(r["out"]).view(np.uint16)
        o = o.reshape(N_GROUPS, 128, 2, 2, 2, 128)      # (G, p, a, s, q, k)
        o = o.transpose(0, 3, 1, 4, 2, 5).reshape(B_SHARD, J)
        outs.append(o)
    out = np.concatenate(outs, axis=0).view(BF16).astype(np.float32)
    return out.reshape(BATCH, J, 1)


# revision 14
# speedup vs baseline: 1.8614x; 1.3260x over previous
"""Trainium2 Bass kernel for ComplexDFT256 — bf16 radix-2 version.

Math: the 256-point complex DFT out = z @ M (z = [xr | xi], M the
512x512 real form of the DFT) is split radix-2 over time samples:
  X[k]      = E[k] + G[k]        k = 0..127
  X[k+128]  = E[k] - G[k]
with E = DFT-128 of even samples and G = (twiddle * DFT-128) of odd
samples; the twiddles fold into G's matrix on the host, so on-device
this is two [B,256]@[256,256] matmuls (half the PE work of the dense
form) plus one add + one sub per output tile (DVE).

Everything streams in bf16 (inputs pre-cast on host, outputs cast by
the DVE butterfly, PSUM accumulates fp32), halving HBM traffic vs
fp32r: 8 MB in + 8 MB out per core.  Measured end-to-end error vs the
fp32 reference ~2.7e-3 of output norm (tolerance 2e-2).

Sharding: pure data parallel over batch across 8 NeuronCores (8192
rows each).  Host pre-permutes columns to [even | odd] order and
transposes to [512, B] so the contraction dim lands on SBUF partitions
with contiguous DMA.

Output leaves the device in a [64, 128, 2, 512] layout (one DMA per
256-row group, partition-major) and is un-permuted on the host.
"""
import numpy as np
import ml_dtypes

import concourse.bacc as bacc
import concourse.mybir as mybir
import concourse.tile as tile
from concourse.bass_utils import run_bass_kernel_spmd

N_CORES = 8
BATCH = 65536
FFT = 256
C = 2 * FFT            # contraction dim = 512 ([even 256 | odd 256])
J = 2 * FFT            # output features = 512
B_SHARD = BATCH // N_CORES   # 8192
CHUNK_B = 2048         # batch rows loaded per DMA chunk
N_CHUNKS = B_SHARD // CHUNK_B
GROUP_B = 512          # batch rows per matmul group (moving free dim)
GROUPS_PER_CHUNK = CHUNK_B // GROUP_B     # 4
N_GROUPS = B_SHARD // GROUP_B             # 16

BF16 = ml_dtypes.bfloat16

_cache = {}


def _build_nc(reps: int = 1, unroll: bool = False):
    nc = bacc.Bacc("TRN2", target_bir_lowering=False, debug=False,
                   num_devices=N_CORES)
    f32 = mybir.dt.float32
    bf16 = mybir.dt.bfloat16

    zt_dram = nc.dram_tensor("zt", [C, B_SHARD], bf16, kind="ExternalInput")
    m_dram = nc.dram_tensor("m", [C, 256], bf16, kind="ExternalInput")
    # transposed output: [group, j-partition, lo/hi, Re/Im, batch-in-group];
    # host un-permutes
    out_dram = nc.dram_tensor("out", [N_GROUPS, 128, 2, 2, GROUP_B], bf16,
                              kind="ExternalOutput")

    with tile.TileContext(nc) as tc:
        with (
            tc.tile_pool(name="mpool", bufs=1) as mpool,
            tc.tile_pool(name="zpool", bufs=3) as zpool,
            tc.tile_pool(name="gpool", bufs=4) as gpool,
            tc.tile_pool(name="opool", bufs=6) as opool,
            tc.tile_pool(name="psum", bufs=2, space="PSUM") as psum_pool,
        ):
            m_sb = []
            for k in range(4):
                mt = mpool.tile([128, 256], bf16, tag=f"m{k}")
                # SWDGE: keeps the m loads off the SP queue so the first
                # zt chunk streams in parallel
                nc.gpsimd.dma_start(mt[:], m_dram[k * 128:(k + 1) * 128, :])
                m_sb.append(mt)

            def body():
                for i in range(N_CHUNKS):
                    zt_sb = zpool.tile([128, 4, CHUNK_B], bf16, tag="zt")
                    for k in range(4):
                        nc.sync.dma_start(
                            zt_sb[:, k, :],
                            zt_dram[k * 128:(k + 1) * 128,
                                    i * CHUNK_B:(i + 1) * CHUNK_B],
                        )
                    for g in range(GROUPS_PER_CHUNK):
                        # Transposed matmuls: stationary = 128x128 M
                        # block, moving = 512 batch columns.  Y[:, 2q+jt]
                        # = [128 j, 512 b] with q=0 even-half (E), q=1
                        # odd-half (G), jt=0 Re / jt=1 Im columns.
                        Y = psum_pool.tile([128, 4, GROUP_B], f32, tag="acc")
                        csl = slice(g * GROUP_B, (g + 1) * GROUP_B)
                        for q in range(2):
                            for jt in range(2):
                                jsl = slice(jt * 128, (jt + 1) * 128)
                                nc.tensor.matmul(
                                    Y[:, 2 * q + jt, :],
                                    m_sb[2 * q][:, jsl],
                                    zt_sb[:, 2 * q, csl],
                                    start=True, stop=False)
                                nc.tensor.matmul(
                                    Y[:, 2 * q + jt, :],
                                    m_sb[2 * q + 1][:, jsl],
                                    zt_sb[:, 2 * q + 1, csl],
                                    start=False, stop=True)
                        # one contiguous PSUM->SBUF drain on ACT (bf16),
                        # so both DVE butterfly ops run all-SBUF in bf16
                        # (DVE TensorTensor may read only one PSUM input,
                        # and all-SBUF 16-bit ops unlock DVE 2x mode)
                        stg = gpool.tile([128, 4, GROUP_B], bf16, tag="stg")
                        nc.scalar.copy(stg[:], Y[:])
                        te = stg[:, 0:2, :]
                        tg = stg[:, 2:4, :]
                        # out_sb[:, a, jt, :]: a=0 -> X[k]=E+G, a=1 ->
                        # X[k+128]=E-G; contiguous 1024-elem DVE writes
                        out_sb = opool.tile([128, 2, 2, GROUP_B], bf16,
                                            tag="out")
                        nc.vector.tensor_add(out_sb[:, 0, :, :], te, tg)
                        nc.vector.tensor_sub(out_sb[:, 1, :, :], te, tg)
                        G = i * GROUPS_PER_CHUNK + g
                        # stores on the SP HWDGE queue with the loads;
                        # ACT stays a pure PSUM-drain engine
                        nc.sync.dma_start(out_dram[G], out_sb[:])

            if reps == 1:
                body()
            elif unroll:
                for _ in range(reps):
                    body()
            else:
                with tc.For_i(0, reps, 1):
                    body()

    nc.compile()
    return nc


def _get_nc():
    if "nc" not in _cache:
        _cache["nc"] = _build_nc()
    return _cache["nc"]


def _prepare_in_maps(x, cos_kernel, sin_kernel):
    x = np.asarray(x, dtype=np.float32)
    cos = np.asarray(cos_kernel, dtype=np.float32)
    sin = np.asarray(sin_kernel, dtype=np.float32)

    m = np.empty((C, J), dtype=np.float32)
    m[:FFT, :FFT] = cos.T
    m[:FFT, FFT:] = sin.T
    m[FFT:, :FFT] = -sin.T
    m[FFT:, FFT:] = cos.T

    # radix-2: even/odd sample rows; cols k<128 of both Re and Im halves
    # (cols k+128 equal these up to the sign of the odd-row block)
    rows_e = np.concatenate([np.arange(0, 256, 2), np.arange(256, 512, 2)])
    rows_o = rows_e + 1
    cols_lo = np.concatenate([np.arange(0, 128), np.arange(256, 384)])
    me = m[np.ix_(rows_e, cols_lo)]     # [256, 256]
    mg = m[np.ix_(rows_o, cols_lo)]     # [256, 256]
    m_dev = np.concatenate([me, mg], axis=0).astype(BF16)  # [512, 256]

    z = x.reshape(BATCH, C)[:, np.concatenate([rows_e, rows_o])].astype(BF16)
    zt = np.ascontiguousarray(z.view(np.uint16).T)  # [512, BATCH] as u16

    in_maps = []
    for c in range(N_CORES):
        shard = np.ascontiguousarray(
            zt[:, c * B_SHARD:(c + 1) * B_SHARD]).view(BF16)
        in_maps.append({"zt": shard, "m": m_dev})
    return in_maps


def _run(in_maps, trace=False):
    nc = _get_nc()
    return run_bass_kernel_spmd(nc, in_maps, list(range(N_CORES)), trace=trace)


def kernel(x, cos_kernel, sin_kernel):
    in_maps = _prepare_in_maps(x, cos_kernel, sin_kernel)
    res = _run(in_maps)
    outs = []
    for r in res.results:
        # [G, p, a, q, b] bf16 (transposed): row = G*GROUP_B + b,
        # col = q*256 + a*128 + p
        o = np.asarray(r["out"]).view(np.uint16)
        o = o.transpose(0, 4, 3, 2, 1).reshape(B_SHARD, J)  # (G,b,q,a,p)
        outs.append(o)
    out = np.concatenate(outs, axis=0).view(BF16).astype(np.float32)
    return out.reshape(BATCH, J, 1)
